# revision 1
# baseline (speedup 1.0000x reference)
"""CenterNet loss kernel for Trainium2 (Bass/Tile), data-parallel over 8 NeuronCores.

Contract: kernel(**inputs) takes the FULL unsharded inputs (numpy arrays, keyed
as in the problem's setup_inputs()) and returns the FULL scalar output.

Strategy (hardcoded for B=16, C=80, H=W=128, K=128, 8 cores):
  - Shard every tensor along batch: 2 batch items per core.
  - Per core, the focal loss over 3 heatmap pairs (63MB of fp32) is streamed
    in [128, 2048] tiles. ScalarE computes e=exp(x), L=ln(1+e) (softplus),
    q=exp(-L), p^2=Square(q-1) from the natural_log_exp table set (one table
    load); GpSimd computes t=x-L; VectorE runs z=p2*L (native bf16 2x) plus
    runtime-registered fused custom DVE ops that accumulate per-partition,
    per-map:
       accS = sum( -(1-y)^4 * L * p^2 )                          (= NL)
       accP = sum( pos ? (x-L)*(1-p)^2 + BIG : 0 )               (= PL + BIG*num_pos)
    where pos = (y > 0.999) is exact (gt values are either 1.0 or < 0.98)
    and BIG=32768 encodes the positive count alongside the (tiny) PL sum.
  - Gathers (embeddings/regressions at gt indices) via iota/is_equal one-hot
    matmuls on the otherwise idle PE engine. The native TensorTensorReduce
    instruction hard-crashes this device at runtime, so all fused
    multiply-reduce steps use the custom-DVE TENSOR_TENSOR_REDUCE op
    instead; smooth-L1 and the AE pull/push partial sums run on tiny tiles.
  - Each core writes a [128, 16] tile of partial sums; the host combines the
    8 tiles with the exact scalar formulas (divisions by num_pos, num, etc).
"""

import os
import sys
from operator import add as _op_add

import numpy as np

if "/opt/trn_rl_repo" not in sys.path:
    sys.path.insert(0, "/opt/trn_rl_repo")

import concourse.bass as bass
import concourse.mybir as mybir
import concourse.tile as tile
from concourse import bacc
from concourse.bass_utils import run_bass_kernel_spmd

F32 = mybir.dt.float32
BF16 = mybir.dt.bfloat16
I32 = mybir.dt.int32
AF = mybir.ActivationFunctionType
ALU = mybir.AluOpType

EPS = 1e-4
N_CORES = 8
B, C, H, W, K = 16, 80, 128, 128, 128
B_PC = B // N_CORES
HWPLANE = H * W

# results tile column map
RC_S = {0: 0, 1: 2, 2: 4}       # per-map acc_S column
RC_NP = {0: 1, 1: 3, 2: 5}      # per-map acc_np column
RC_L1 = 6
RC_NUM = 8    # + b (row 0)
RC_PULL = 10  # + b (row 0)
RC_S1 = 12    # + b (full column)
RC_S2 = 14    # + b (full column)
RES_COLS = 16

_custom_ops_cache = {}


def _get_custom_ops():
    """Register the fused DVE ops this kernel uses (process-local, additive —
    the same mechanism dve_ops.py documents for adding ops)."""
    if _custom_ops_cache:
        return _custom_ops_cache
    import concourse.dve_ops as dve_ops
    from concourse.dve_spec import C0, C1, C2, One, Spec, Src0, Src1, Zero, lower, select, sq
    from concourse.dve_uop import DveOpSpec

    def _sum_keep(b):
        return b.reshape(b.shape[0], -1).sum(axis=-1, keepdims=True)

    def _mk(name, spec):
        uops = lower(spec, ver="v3")
        sha = DveOpSpec(name=name, opcode=1, uops=uops,
                        rd1_en=dve_ops.has_src1(spec)).sha("v3")
        return dve_ops.DveOp(name, spec, subdim=False, uops_sha={"v3": sha})

    specs = {
        # h = t * q^2
        "CNL_MUL_SQ": Spec(
            body=Src0 * sq(Src1),
            reference=lambda in0, in1, c0, c1, c2: (
                in0.astype(np.float32) * np.square(in1.astype(np.float32))
            ),
        ),
        # z = (c0 - q)^2 * L
        "CNL_SQ1M_MUL": Spec(
            body=sq(C0 - Src0) * Src1,
            reference=lambda in0, in1, c0, c1, c2: (
                np.square(c0 - in0.astype(np.float32)) * in1.astype(np.float32)
            ),
        ),
        # acc = c1 + sum( (c0 < y) ? h : 0 )
        "CNL_SEL_ACC": Spec(
            body=select(C0 < Src1, Src0, Zero),
            accum=_op_add,
            accum_init=C1,
            reference=lambda in0, in1, c0, c1, c2: (
                lambda b: (b, c1 + b.reshape(b.shape[0], -1).sum(-1, keepdims=True))
            )(np.where(c0 < in1.astype(np.float32), in0.astype(np.float32), 0.0).astype(np.float32)),
        ),
        # acc = c1 + sum( ((y - c0)^2)^2 * z * imm2 )
        "CNL_QUART_ACC": Spec(
            body=sq(sq(Src0 - C0)) * Src1 * C2,
            accum=_op_add,
            accum_init=C1,
            reference=lambda in0, in1, c0, c1, c2: (
                lambda b: (b, c1 + b.reshape(b.shape[0], -1).sum(-1, keepdims=True))
            )((np.square(np.square(in0.astype(np.float32) - c0)) * in1.astype(np.float32) * c2).astype(np.float32)),
        ),
        # acc = c1 + sum( (c0 < y) ? 1 : 0 )
        "CNL_CNT_ACC": Spec(
            body=select(C0 < Src0, One, Zero),
            accum=_op_add,
            accum_init=C1,
            reference=lambda in0, in1, c0, c1, c2: (
                lambda b: (b, c1 + b.reshape(b.shape[0], -1).sum(-1, keepdims=True))
            )(np.where(c0 < in0.astype(np.float32), 1.0, 0.0).astype(np.float32)),
        ),
        # acc = c1 + sum( (c0 < y) ? h + imm2 : 0 )  — pos-masked sum with the
        # positive count encoded at offset imm2 (=BIG) in the same accumulator
        "CNL_SELBIG_ACC": Spec(
            body=select(C0 < Src1, Src0 + C2, Zero),
            accum=_op_add,
            accum_init=C1,
            reference=lambda in0, in1, c0, c1, c2: (
                lambda b: (b, c1 + b.reshape(b.shape[0], -1).sum(-1, keepdims=True))
            )(np.where(c0 < in1.astype(np.float32), in0.astype(np.float32) + c2, 0.0).astype(np.float32)),
        ),
    }
    existing = {op.name for op in dve_ops.OPS}
    made = {}
    for name, spec in specs.items():
        if name not in existing:
            op = _mk(name, spec)
            dve_ops.OPS.append(op)
        else:
            op = next(o for o in dve_ops.OPS if o.name == name)
        made[name] = op
    # rebuild the name->row map and CoreSim spec registry
    dve_ops._SUB_OPCODE_FOR_NAME.clear()
    dve_ops._SUB_OPCODE_FOR_NAME.update(
        {op.name: dve_ops._CUSTOM_DVE_ROW_BASE + i for i, op in enumerate(dve_ops.OPS)}
    )
    assert max(dve_ops._SUB_OPCODE_FOR_NAME.values()) < 0x20
    dve_ops.CUSTOM_DVE_SPECS.update({op.name: op.spec for op in dve_ops.OPS})
    made["TTR"] = next(o for o in dve_ops.OPS if o.name == "TENSOR_TENSOR_REDUCE")
    _custom_ops_cache.update(made)
    return _custom_ops_cache


BIG = 32768.0  # np-count encoding offset for the fused pos-select op


def build_module(b_pc=B_PC, c_maps=C, tile_f=2048, use_custom=True, trn_type="TRN2",
                 version=3, skip_focal=False, skip_small=False, small_stage=6):
    """Build the per-core Bass module. Returns nc."""
    ops = _get_custom_ops() if use_custom else None

    nc = bacc.Bacc(trn_type, target_bir_lowering=False)
    nf = b_pc * c_maps * HWPLANE // 128
    n_tiles = nf // tile_f
    assert n_tiles * tile_f == nf

    maps = ["tl", "br", "ct"]
    hm, gt, embd, regs, gtr, inds = {}, {}, {}, {}, {}, {}
    for s in maps:
        hm[s] = nc.declare_dram_parameter(f"hmap_{s}", [128, nf], F32, isOutput=False)
        gt[s] = nc.declare_dram_parameter(f"gt_hmap_{s}", [128, nf], F32, isOutput=False)
    for s in ("tl", "br"):
        embd[s] = nc.declare_dram_parameter(f"embd_{s}", [b_pc, 128, 128], F32, isOutput=False)
    for s in maps:
        regs[s] = nc.declare_dram_parameter(f"regs_{s}", [b_pc, 2, 128, 128], F32, isOutput=False)
        gtr[s] = nc.declare_dram_parameter(f"gt_regs_{s}", [b_pc, 128, 2], F32, isOutput=False)
        inds[s] = nc.declare_dram_parameter(f"inds_{s}", [b_pc, 128, 1], I32, isOutput=False)
    masks_p = nc.declare_dram_parameter("ind_masks", [b_pc, 128, 1], I32, isOutput=False)
    out_p = nc.declare_dram_parameter("out", [128, RES_COLS], F32, isOutput=True)

    from contextlib import ExitStack

    with tile.TileContext(nc) as tc, ExitStack() as ctx:
        const_pool = ctx.enter_context(tc.tile_pool(name="const", bufs=1))
        acc_pool = ctx.enter_context(tc.tile_pool(name="acc", bufs=1))
        big_pool = ctx.enter_context(tc.tile_pool(name="big", bufs=3))
        # v7's bf16 intermediates halve the mid-pool footprint — spend it on a
        # third buffer set for deeper DMA/ACT/DVE/GpSimd overlap
        mid_pool = ctx.enter_context(tc.tile_pool(name="mid", bufs=3 if version >= 7 else 2))
        small_pool = ctx.enter_context(tc.tile_pool(name="small", bufs=2))
        psum_pool = ctx.enter_context(tc.tile_pool(name="ps", bufs=2, space="PSUM"))

        # ---- constants -------------------------------------------------
        iota_fi = const_pool.tile([128, 128], I32, tag="iota_fi", name="iota_fi")
        nc.gpsimd.iota(iota_fi[:], pattern=[[1, 128]], base=0, channel_multiplier=0)
        iota_pi = const_pool.tile([128, 1], I32, tag="iota_pi", name="iota_pi")
        nc.gpsimd.iota(iota_pi[:], pattern=[[1, 1]], base=0, channel_multiplier=1)
        iota_f = const_pool.tile([128, 128], F32, tag="iota_f", name="iota_f")
        nc.vector.tensor_copy(iota_f[:], iota_fi[:])
        iota_p = const_pool.tile([128, 1], F32, tag="iota_p", name="iota_p")
        nc.vector.tensor_copy(iota_p[:], iota_pi[:])
        ident = const_pool.tile([128, 128], F32, tag="ident", name="ident")
        nc.vector.tensor_scalar(ident[:], iota_f[:], iota_p[:], None, ALU.is_equal)
        ones_row = const_pool.tile([1, 128], F32, tag="ones_row", name="ones_row")
        nc.vector.memset(ones_row[:], 1.0)
        ones2 = const_pool.tile([128, 2], F32, tag="ones2", name="ones2")
        nc.vector.memset(ones2[:], 1.0)
        cneg1 = const_pool.tile([128, 1], F32, tag="cneg1", name="cneg1")
        nc.vector.memset(cneg1[:], -1.0)

        results = acc_pool.tile([128, RES_COLS], F32, tag="results", name="results")
        nc.vector.memset(results[:], 0.0)

        # ---- focal loss over the 3 heatmap pairs -----------------------
        accS = {}
        accN = {}
        accP = {}
        for m in range(3):
            accS[m] = [acc_pool.tile([128, 1], F32, tag=f"accS{m}_{i}", name=f"accS{m}_{i}") for i in range(2)]
            nc.vector.memset(accS[m][0][:], 0.0)
            if version >= 3:
                accP[m] = [acc_pool.tile([128, 1], F32, tag=f"accP{m}_{i}", name=f"accP{m}_{i}") for i in range(2)]
                nc.vector.memset(accP[m][0][:], 0.0)
            if version < 3:
                accN[m] = [acc_pool.tile([128, 1], F32, tag=f"accN{m}_{i}", name=f"accN{m}_{i}") for i in range(2)]
                nc.vector.memset(accN[m][0][:], 0.0)

        for m, s in enumerate(maps):
            if skip_focal:
                break
            for j in range(n_tiles):
                sl = slice(j * tile_f, (j + 1) * tile_f)
                xt = big_pool.tile([128, tile_f], F32, tag="xt", name="xt")
                nc.sync.dma_start(out=xt[:], in_=hm[s][:, sl])
                yt = big_pool.tile([128, tile_f], F32, tag="yt", name="yt")
                nc.sync.dma_start(out=yt[:], in_=gt[s][:, sl])

                mdt = BF16 if version >= 7 else F32
                et = mid_pool.tile([128, tile_f], F32, tag="et", name="et")
                nc.scalar.activation(et[:], xt[:], AF.Exp)
                Lt = mid_pool.tile([128, tile_f], mdt, tag="Lt", name="Lt")
                nc.scalar.activation(Lt[:], et[:], AF.Ln, bias=1.0)
                qt = mid_pool.tile([128, tile_f], mdt, tag="qt", name="qt")
                nc.scalar.activation(qt[:], Lt[:], AF.Exp, scale=-1.0)

                if use_custom and version >= 4:
                    # t = x - L on GpSimd (fp32 x, bf16 L -> bf16 t)
                    tt = mid_pool.tile([128, tile_f], BF16, tag="tt", name="tt")
                    nc.gpsimd.tensor_tensor(tt[:], xt[:], Lt[:], ALU.subtract)
                    # p^2 = (q - 1)^2 on ACT
                    p2t = mid_pool.tile([128, tile_f], BF16, tag="p2t", name="p2t")
                    nc.scalar.activation(p2t[:], qt[:], AF.Square, bias=cneg1[:])
                    ht = mid_pool.tile([128, tile_f], BF16, tag="ht", name="ht")
                    nc.vector._custom_dve(ops["CNL_MUL_SQ"], out=ht[:], in0=tt[:], in1=qt[:])
                    zt = mid_pool.tile([128, tile_f], BF16, tag="zt", name="zt")
                    nc.vector.tensor_tensor(zt[:], p2t[:], Lt[:], ALU.mult)
                    sA = mid_pool.tile([128, tile_f], BF16, tag="sA", name="sA")
                    nc.vector._custom_dve(
                        ops["CNL_SELBIG_ACC"], out=sA[:], in0=ht[:], in1=yt[:],
                        s0=0.999, s1=accP[m][j % 2][:], imm2=BIG,
                        accum_out=accP[m][(j + 1) % 2][:],
                    )
                    sB = mid_pool.tile([128, tile_f], BF16, tag="sB", name="sB")
                    nc.vector._custom_dve(
                        ops["CNL_QUART_ACC"], out=sB[:], in0=yt[:], in1=zt[:],
                        s0=1.0, s1=accS[m][j % 2][:], imm2=-1.0,
                        accum_out=accS[m][(j + 1) % 2][:],
                    )
                    continue

                tt = mid_pool.tile([128, tile_f], mdt, tag="tt", name="tt")
                if version >= 5:
                    # t = x - L on the otherwise idle GpSimd engine; VectorE is
                    # the bottleneck (custom ops are 1x-rate only)
                    nc.gpsimd.tensor_tensor(tt[:], xt[:], Lt[:], ALU.subtract)
                else:
                    nc.vector.tensor_tensor(tt[:], xt[:], Lt[:], ALU.subtract)

                if use_custom and version >= 3:
                    ht = mid_pool.tile([128, tile_f], mdt, tag="ht", name="ht")
                    nc.vector._custom_dve(ops["CNL_MUL_SQ"], out=ht[:], in0=tt[:], in1=qt[:])
                    zt = mid_pool.tile([128, tile_f], mdt, tag="zt", name="zt")
                    if version >= 7:
                        # p^2 = Square(q - 1) on ACT, then z = p2 * L as a
                        # native bf16 tensor_tensor (2x mode) — drops the
                        # 1x-rate SQ1M custom op from the DVE critical path
                        p2t = mid_pool.tile([128, tile_f], BF16, tag="p2t", name="p2t")
                        nc.scalar.activation(p2t[:], qt[:], AF.Square, bias=cneg1[:])
                        nc.vector.tensor_tensor(zt[:], p2t[:], Lt[:], ALU.mult)
                    else:
                        nc.vector._custom_dve(ops["CNL_SQ1M_MUL"], out=zt[:], in0=qt[:], in1=Lt[:], s0=1.0)
                    sA = mid_pool.tile([128, tile_f], BF16, tag="sA", name="sA")
                    nc.vector._custom_dve(
                        ops["CNL_SELBIG_ACC"], out=sA[:], in0=ht[:], in1=yt[:],
                        s0=0.999, s1=accP[m][j % 2][:], imm2=BIG,
                        accum_out=accP[m][(j + 1) % 2][:],
                    )
                    sB = mid_pool.tile([128, tile_f], BF16, tag="sB", name="sB")
                    nc.vector._custom_dve(
                        ops["CNL_QUART_ACC"], out=sB[:], in0=yt[:], in1=zt[:],
                        s0=1.0, s1=accS[m][j % 2][:], imm2=-1.0,
                        accum_out=accS[m][(j + 1) % 2][:],
                    )
                    continue

                yb = mid_pool.tile([128, tile_f], BF16, tag="yb", name="yb")
                nc.gpsimd.tensor_copy(yb[:], yt[:])

                if use_custom:
                    ht = mid_pool.tile([128, tile_f], BF16, tag="ht", name="ht")
                    nc.vector._custom_dve(ops["CNL_MUL_SQ"], out=ht[:], in0=tt[:], in1=qt[:])
                    zt = mid_pool.tile([128, tile_f], BF16, tag="zt", name="zt")
                    nc.vector._custom_dve(ops["CNL_SQ1M_MUL"], out=zt[:], in0=qt[:], in1=Lt[:], s0=1.0)
                    sA = mid_pool.tile([128, tile_f], BF16, tag="sA", name="sA")
                    nc.vector._custom_dve(
                        ops["CNL_SEL_ACC"], out=sA[:], in0=ht[:], in1=yb[:],
                        s0=0.999, s1=accS[m][0][:], accum_out=accS[m][1][:],
                    )
                    sB = mid_pool.tile([128, tile_f], BF16, tag="sB", name="sB")
                    nc.vector._custom_dve(
                        ops["CNL_QUART_ACC"], out=sB[:], in0=yb[:], in1=zt[:],
                        s0=1.0, s1=accS[m][1][:], imm2=-1.0, accum_out=accS[m][0][:],
                    )
                    sC = mid_pool.tile([128, tile_f], BF16, tag="sC", name="sC")
                    nc.vector._custom_dve(
                        ops["CNL_CNT_ACC"], out=sC[:], in0=yb[:],
                        s0=0.999, s1=accN[m][j % 2][:], accum_out=accN[m][(j + 1) % 2][:],
                    )
                else:
                    q2 = mid_pool.tile([128, tile_f], BF16, tag="q2", name="q2")
                    nc.vector.tensor_tensor(q2[:], qt[:], qt[:], ALU.mult)
                    ht = mid_pool.tile([128, tile_f], BF16, tag="ht", name="ht")
                    nc.vector.tensor_tensor(ht[:], tt[:], q2[:], ALU.mult)
                    pos = mid_pool.tile([128, tile_f], BF16, tag="pos", name="pos")
                    nc.vector.tensor_scalar(pos[:], yb[:], 1.0, None, ALU.is_equal)
                    sA = mid_pool.tile([128, tile_f], BF16, tag="sA", name="sA")
                    nc.vector.tensor_tensor_reduce(
                        out=sA[:], in0=pos[:], in1=ht[:], scale=1.0,
                        scalar=accS[m][0][:], op0=ALU.mult, op1=ALU.add,
                        accum_out=accS[m][1][:],
                    )
                    pt = mid_pool.tile([128, tile_f], BF16, tag="pt", name="pt")
                    nc.vector.tensor_scalar(pt[:], qt[:], -1.0, 1.0, ALU.mult, ALU.add)
                    p2 = mid_pool.tile([128, tile_f], BF16, tag="p2", name="p2")
                    nc.vector.tensor_tensor(p2[:], pt[:], pt[:], ALU.mult)
                    zt = mid_pool.tile([128, tile_f], BF16, tag="zt", name="zt")
                    nc.vector.tensor_tensor(zt[:], p2[:], Lt[:], ALU.mult)
                    ut = mid_pool.tile([128, tile_f], BF16, tag="ut", name="ut")
                    nc.vector.tensor_scalar(ut[:], yb[:], -1.0, 1.0, ALU.mult, ALU.add)
                    u2 = mid_pool.tile([128, tile_f], BF16, tag="u2", name="u2")
                    nc.vector.tensor_tensor(u2[:], ut[:], ut[:], ALU.mult)
                    u4 = mid_pool.tile([128, tile_f], BF16, tag="u4", name="u4")
                    nc.vector.tensor_tensor(u4[:], u2[:], u2[:], ALU.mult)
                    sB = mid_pool.tile([128, tile_f], BF16, tag="sB", name="sB")
                    nc.vector.tensor_tensor_reduce(
                        out=sB[:], in0=u4[:], in1=zt[:], scale=-1.0,
                        scalar=accS[m][1][:], op0=ALU.mult, op1=ALU.add,
                        accum_out=accS[m][0][:],
                    )
                    sC = mid_pool.tile([128, tile_f], BF16, tag="sC", name="sC")
                    nc.vector.tensor_tensor_reduce(
                        out=sC[:], in0=pos[:], in1=pos[:], scale=1.0,
                        scalar=accN[m][j % 2][:], op0=ALU.mult, op1=ALU.add,
                        accum_out=accN[m][(j + 1) % 2][:],
                    )
            if version >= 3:
                nc.vector.tensor_copy(results[:, RC_S[m]:RC_S[m] + 1], accS[m][n_tiles % 2][:])
                nc.vector.tensor_copy(results[:, RC_NP[m]:RC_NP[m] + 1], accP[m][n_tiles % 2][:])
            else:
                nc.vector.tensor_copy(results[:, RC_S[m]:RC_S[m] + 1], accS[m][0][:])
                nc.vector.tensor_copy(results[:, RC_NP[m]:RC_NP[m] + 1], accN[m][n_tiles % 2][:])

        # ---- gathers + smooth-L1 + AE ---------------------------------
        accL1 = [acc_pool.tile([128, 1], F32, tag=f"accL1_{i}", name=f"accL1_{i}") for i in range(2)]
        nc.vector.memset(accL1[0][:], 0.0)
        l1_step = 0

        for b in range(b_pc if not skip_small else 0):
            mask_i = small_pool.tile([128, 1], I32, tag="mask_i", name="mask_i")
            nc.sync.dma_start(out=mask_i[:], in_=masks_p[b])
            mask_f = small_pool.tile([128, 1], F32, tag="mask_f", name="mask_f")
            nc.vector.tensor_copy(mask_f[:], mask_i[:])

            ae_in = small_pool.tile([128, 4], F32, tag="ae_in", name="ae_in")
            nc.vector.memset(ae_in[:], 0.0)
            nc.vector.tensor_copy(ae_in[:, 2:3], mask_f[:])

            if small_stage < 2:
                continue
            for m, s in enumerate(maps):
                has_embd = s in ("tl", "br")
                ind_t = small_pool.tile([128, 1], I32, tag="ind_t", name="ind_t")
                nc.sync.dma_start(out=ind_t[:], in_=inds[s][b])
                r_ti = small_pool.tile([128, 1], I32, tag="r_ti", name="r_ti")
                nc.vector.tensor_scalar(r_ti[:], ind_t[:], 7, None, ALU.arith_shift_right)
                c_ti = small_pool.tile([128, 1], I32, tag="c_ti", name="c_ti")
                nc.vector.tensor_scalar(c_ti[:], ind_t[:], 127, None, ALU.bitwise_and)
                r_t = small_pool.tile([128, 1], F32, tag="r_t", name="r_t")
                nc.vector.tensor_copy(r_t[:], r_ti[:])
                c_t = small_pool.tile([128, 1], F32, tag="c_t", name="c_t")
                nc.vector.tensor_copy(c_t[:], c_ti[:])
                ohRT = small_pool.tile([128, 128], F32, tag="ohRT", name="ohRT")
                nc.vector.tensor_scalar(ohRT[:], iota_f[:], r_t[:], None, ALU.is_equal)
                ohCT = small_pool.tile([128, 128], F32, tag="ohCT", name="ohCT")
                nc.vector.tensor_scalar(ohCT[:], iota_f[:], c_t[:], None, ALU.is_equal)
                if small_stage < 3:
                    continue
                psT = psum_pool.tile([128, 128], F32, tag="psT", name="psT")
                nc.tensor.transpose(psT[:], ohRT[:], ident[:])
                ohR = small_pool.tile([128, 128], F32, tag="ohR", name="ohR")
                nc.vector.tensor_copy(ohR[:], psT[:])

                if small_stage < 4:
                    continue
                gcols = 384 if has_embd else 256
                off = 128 if has_embd else 0
                plane = small_pool.tile([128, 384], F32, tag="plane", name="plane")
                if has_embd:
                    nc.sync.dma_start(out=plane[:, 0:128], in_=embd[s][b])
                nc.sync.dma_start(out=plane[:, off:off + 128], in_=regs[s][b, 0])
                nc.sync.dma_start(out=plane[:, off + 128:off + 256], in_=regs[s][b, 1])

                psG = psum_pool.tile([128, 384], F32, tag="psG", name="psG")
                nc.tensor.matmul(psG[:, 0:gcols], lhsT=ohR[:], rhs=plane[:, 0:gcols],
                                 start=True, stop=True)

                scr = small_pool.tile([128, 128], F32, tag="scr", name="scr")
                if has_embd:
                    col = 0 if s == "tl" else 1
                    nc.vector._custom_dve(
                        ops["TTR"], out=scr[:], in0=psG[:, 0:128], in1=ohCT[:],
                        s0=0.0, s1=1.0, accum_out=ae_in[:, col:col + 1],
                    )
                rg = small_pool.tile([128, 2], F32, tag="rg", name="rg")
                for ch in range(2):
                    o = off + ch * 128
                    nc.vector._custom_dve(
                        ops["TTR"], out=scr[:], in0=psG[:, o:o + 128], in1=ohCT[:],
                        s0=0.0, s1=1.0, accum_out=rg[:, ch:ch + 1],
                    )

                if small_stage < 5:
                    continue
                gtg = small_pool.tile([128, 2], F32, tag="gtg", name="gtg")
                nc.sync.dma_start(out=gtg[:], in_=gtr[s][b])
                d = small_pool.tile([128, 2], F32, tag="d", name="d")
                nc.vector.tensor_tensor(d[:], rg[:], gtg[:], ALU.subtract)
                ad = small_pool.tile([128, 2], F32, tag="ad", name="ad")
                nc.scalar.activation(ad[:], d[:], AF.Abs)
                mn = small_pool.tile([128, 2], F32, tag="mn", name="mn")
                nc.vector.tensor_scalar(mn[:], ad[:], 1.0, None, ALU.min)
                t1 = small_pool.tile([128, 2], F32, tag="t1", name="t1")
                nc.vector.tensor_scalar(t1[:], ad[:], 2.0, None, ALU.mult)
                w1 = small_pool.tile([128, 2], F32, tag="w1", name="w1")
                nc.vector.tensor_tensor(w1[:], t1[:], mn[:], ALU.subtract)
                smk = small_pool.tile([128, 2], F32, tag="smk", name="smk")
                nc.vector.tensor_scalar(smk[:], w1[:], mask_f[:], None, ALU.mult)
                sm = small_pool.tile([128, 2], F32, tag="sm", name="sm")
                nc.vector.tensor_tensor(sm[:], smk[:], mn[:], ALU.mult)
                scr2 = small_pool.tile([128, 2], F32, tag="scr2", name="scr2")
                nc.vector._custom_dve(
                    ops["TTR"], out=scr2[:], in0=sm[:], in1=ones2[:],
                    s0=accL1[l1_step % 2][:], s1=0.5,
                    accum_out=accL1[(l1_step + 1) % 2][:],
                )
                l1_step += 1

            if small_stage < 6:
                continue
            # ---- AE loss for this batch item --------------------------
            # transpose e0/e1/mask columns into rows of a single-partition
            # [1, 384] psum tile (PSUM reads must start at partition 0)
            psA = psum_pool.tile([1, 384], F32, tag="psA", name="psA")
            for ci in range(3):
                nc.tensor.transpose(psA[0:1, ci * 128:(ci + 1) * 128],
                                    ae_in[:, ci:ci + 1], ident[:])
            aeR = small_pool.tile([1, 384], F32, tag="aeR", name="aeR")
            nc.vector.tensor_copy(aeR[:], psA[:])
            s_row = small_pool.tile([1, 128], F32, tag="s_row", name="s_row")
            nc.vector.tensor_tensor(s_row[:], aeR[0:1, 0:128], aeR[0:1, 128:256], ALU.add)
            bc_in = small_pool.tile([1, 256], F32, tag="bc_in", name="bc_in")
            nc.vector.tensor_scalar(bc_in[0:1, 0:128], s_row[:], 0.5, None, ALU.mult)
            nc.vector.tensor_copy(bc_in[0:1, 128:256], aeR[0:1, 256:384])
            d_row = small_pool.tile([1, 128], F32, tag="d_row", name="d_row")
            nc.vector.tensor_tensor(d_row[:], aeR[0:1, 0:128], aeR[0:1, 128:256], ALU.subtract)
            sc = small_pool.tile([128, 1], F32, tag="sc", name="sc")
            nc.vector.tensor_tensor(sc[:], ae_in[:, 0:1], ae_in[:, 1:2], ALU.add)
            mc = small_pool.tile([128, 1], F32, tag="mc", name="mc")
            nc.vector.tensor_scalar(mc[:], sc[:], 0.5, None, ALU.mult)

            psB = psum_pool.tile([128, 256], F32, tag="psB", name="psB")
            nc.tensor.matmul(psB[:], lhsT=ones_row[:], rhs=bc_in[:], start=True, stop=True)

            Mm = small_pool.tile([128, 128], F32, tag="Mm", name="Mm")
            nc.vector.tensor_scalar(Mm[:], psB[:, 0:128], mc[:], None, ALU.subtract)
            Aa = small_pool.tile([128, 128], F32, tag="Aa", name="Aa")
            nc.scalar.activation(Aa[:], Mm[:], AF.Abs)
            Dd = small_pool.tile([128, 128], F32, tag="Dd", name="Dd")
            nc.vector.tensor_scalar(Dd[:], Aa[:], -1.0, 1.0, ALU.mult, ALU.add)
            pm = small_pool.tile([128, 128], F32, tag="pm", name="pm")
            nc.vector.tensor_scalar(pm[:], psB[:, 128:256], mask_f[:], None, ALU.mult)
            Dr = small_pool.tile([128, 128], F32, tag="Dr", name="Dr")
            nc.vector.tensor_scalar(Dr[:], Dd[:], 0.0, None, ALU.max)
            scrP = small_pool.tile([128, 128], F32, tag="scrP", name="scrP")
            nc.vector._custom_dve(
                ops["TTR"], out=scrP[:], in0=Dr[:], in1=pm[:],
                s0=0.0, s1=1.0,
                accum_out=results[:, RC_S1 + b:RC_S1 + b + 1],
            )
            scrQ = small_pool.tile([128, 128], F32, tag="scrQ", name="scrQ")
            nc.vector.tensor_scalar(
                scrQ[:], pm[:], 0.0, 0.0, ALU.add, ALU.add,
                accum_out=results[:, RC_S2 + b:RC_S2 + b + 1],
            )
            sdm = small_pool.tile([1, 128], F32, tag="sdm", name="sdm")
            nc.vector.tensor_tensor(sdm[:], d_row[:], bc_in[0:1, 128:256], ALU.mult)
            scrR = small_pool.tile([1, 128], F32, tag="scrR", name="scrR")
            nc.vector._custom_dve(
                ops["TTR"], out=scrR[:], in0=sdm[:], in1=d_row[:],
                s0=0.0, s1=0.5,
                accum_out=results[0:1, RC_PULL + b:RC_PULL + b + 1],
            )
            scrN = small_pool.tile([1, 128], F32, tag="scrN", name="scrN")
            nc.vector.tensor_scalar(
                scrN[:], bc_in[0:1, 128:256], 0.0, 0.0, ALU.add, ALU.add,
                accum_out=results[0:1, RC_NUM + b:RC_NUM + b + 1],
            )

        nc.vector.tensor_copy(results[:, RC_L1:RC_L1 + 1], accL1[l1_step % 2][:])
        nc.sync.dma_start(out=out_p[:], in_=results[:])

    nc.compile()
    return nc


def shard_inputs(inputs, b_pc=B_PC, n_cores=N_CORES):
    """Slice full inputs into per-core input maps (named as dram params)."""
    in_maps = []
    for core in range(n_cores):
        b0, b1 = core * b_pc, (core + 1) * b_pc
        m = {}
        for s in ("tl", "br", "ct"):
            m[f"hmap_{s}"] = np.ascontiguousarray(inputs[f"hmap_{s}"][b0:b1]).reshape(128, -1)
            m[f"gt_hmap_{s}"] = np.ascontiguousarray(inputs[f"gt_hmap_{s}"][b0:b1]).reshape(128, -1)
            m[f"regs_{s}"] = np.ascontiguousarray(inputs[f"regs_{s}"][b0:b1]).reshape(b_pc, 2, 128, 128)
            m[f"gt_regs_{s}"] = np.ascontiguousarray(inputs[f"gt_regs_{s}"][b0:b1]).reshape(b_pc, 128, 2)
            m[f"inds_{s}"] = np.ascontiguousarray(inputs[f"inds_{s}"][b0:b1]).reshape(b_pc, 128, 1)
        for s in ("tl", "br"):
            m[f"embd_{s}"] = np.ascontiguousarray(inputs[f"embd_{s}"][b0:b1]).reshape(b_pc, 128, 128)
        m["ind_masks"] = np.ascontiguousarray(inputs["ind_masks"][b0:b1]).reshape(b_pc, 128, 1)
        in_maps.append(m)
    return in_maps


def host_combine(res_list, b_pc=B_PC, fused_np=True):
    """Combine per-core [128,16] partial-sum tiles into the scalar loss."""
    res = [np.asarray(r, dtype=np.float64) for r in res_list]
    focal = 0.0
    for m in range(3):
        if fused_np:
            v = np.concatenate([r[:, RC_NP[m]] for r in res])
            np_p = np.round(v / BIG)
            PL = (v - np_p * BIG).sum()
            NP = np_p.sum()
            S = PL + sum(r[:, RC_S[m]].sum() for r in res)
        else:
            S = sum(r[:, RC_S[m]].sum() for r in res)
            NP = sum(r[:, RC_NP[m]].sum() for r in res)
        focal += (-S / NP) if NP > 0 else -S
    l1 = sum(r[:, RC_L1].sum() for r in res)
    num_global = 0.0
    pull = 0.0
    push = 0.0
    for r in res:
        for b in range(b_pc):
            num_b = r[0, RC_NUM + b]
            num_global += num_b
            pull += r[0, RC_PULL + b] / (num_b + EPS)
            S1 = r[:, RC_S1 + b].sum()
            S2 = r[:, RC_S2 + b].sum()
            push += (S1 - S2 / (num_b + EPS)) / ((num_b - 1.0) * num_b + EPS)
    reg = l1 / (num_global + EPS)
    total = focal + 0.1 * pull + 0.1 * push + reg
    return np.float32(total)


_nc_cache = {}
last_exec_time_ns = None


VERSION = int(os.environ.get("CNL_VERSION", "7"))


def kernel(**inputs):
    global last_exec_time_ns
    if "nc" not in _nc_cache:
        _nc_cache["nc"] = build_module(version=VERSION)
    nc = _nc_cache["nc"]
    in_maps = shard_inputs(inputs)
    trace = bool(int(os.environ.get("CNL_TRACE", "0")))
    kr = run_bass_kernel_spmd(nc, in_maps, core_ids=list(range(N_CORES)), trace=trace)
    last_exec_time_ns = kr.exec_time_ns
    return host_combine([r["out"] for r in kr.results], fused_np=(VERSION >= 3))



# revision 2
# speedup vs baseline: 209.9782x; 209.9782x over previous
"""CenterNet loss kernel for Trainium2 (Bass/Tile), data-parallel over 8 NeuronCores.

Contract: kernel(**inputs) takes the FULL unsharded inputs (numpy arrays, keyed
as in the problem's setup_inputs()) and returns the FULL scalar output.

Strategy (hardcoded for B=16, C=80, H=W=128, K=128, 8 cores):
  - Shard every tensor along batch: 2 batch items per core.
  - Per core, the focal loss over 3 heatmap pairs is streamed in [128, 2048]
    tiles. ScalarE computes e=exp(x), L=ln(1+e) (softplus), q=exp(-L),
    p^2=Square(q-1) from the natural_log_exp table set (one table load);
    GpSimd computes t=x-L; VectorE runs z=p2*L (native bf16 2x) plus
    runtime-registered fused custom DVE ops that accumulate per-partition,
    per-map:
       accS = sum( -(1-y)^4 * L * p^2 )                          (= NL)
       accP = sum( pos ? (x-L)*(1-p)^2 + BIG : 0 )               (= PL + BIG*num_pos)
    where pos = (y > 0.999) is exact (gt values are either 1.0 or < 0.98)
    and BIG=32768 encodes the positive count alongside the (tiny) PL sum.
  - Gathers (embeddings/regressions at gt indices) via iota/is_equal one-hot
    matmuls on the otherwise idle PE engine. The native TensorTensorReduce
    instruction hard-crashes this device at runtime, so all fused
    multiply-reduce steps use the custom-DVE TENSOR_TENSOR_REDUCE op
    instead; smooth-L1 and the AE pull/push partial sums run on tiny tiles.
  - Each core writes a [128, 16] tile of partial sums; the host combines the
    8 tiles with the exact scalar formulas (divisions by num_pos, num, etc).

build_module(reps=K) wraps the whole per-core computation in a hardware
For_i loop that re-runs it K times (inputs re-read from DRAM each rep,
accumulators re-zeroed, output rewritten). reps>1 exists purely so a test
harness can isolate per-execution device time as
(wall(K) - wall(1)) / (K - 1) — the axon-tunneled dispatch overhead
(~80 ms/call on this rig, independent of kernel content) cancels in the
difference. kernel() itself always uses reps=1.
"""

import os
import sys
from operator import add as _op_add

import numpy as np

if "/opt/trn_rl_repo" not in sys.path:
    sys.path.insert(0, "/opt/trn_rl_repo")

import concourse.bass as bass
import concourse.mybir as mybir
import concourse.tile as tile
from concourse import bacc
from concourse.bass_utils import run_bass_kernel_spmd

F32 = mybir.dt.float32
BF16 = mybir.dt.bfloat16
I32 = mybir.dt.int32
AF = mybir.ActivationFunctionType
ALU = mybir.AluOpType

EPS = 1e-4
N_CORES = 8
B, C, H, W, K = 16, 80, 128, 128, 128
B_PC = B // N_CORES
HWPLANE = H * W

# results tile column map
RC_S = {0: 0, 1: 2, 2: 4}       # per-map acc_S column
RC_NP = {0: 1, 1: 3, 2: 5}      # per-map acc_np column
RC_L1 = 6
RC_NUM = 8    # + b (row 0)
RC_PULL = 10  # + b (row 0)
RC_S1 = 12    # + b (full column)
RC_S2 = 14    # + b (full column)
RES_COLS = 16

_custom_ops_cache = {}


def _get_custom_ops():
    """Register the fused DVE ops this kernel uses (process-local, additive —
    the same mechanism dve_ops.py documents for adding ops)."""
    if _custom_ops_cache:
        return _custom_ops_cache
    import concourse.dve_ops as dve_ops
    from concourse.dve_spec import C0, C1, C2, One, Spec, Src0, Src1, Zero, lower, select, sq
    from concourse.dve_uop import DveOpSpec

    def _mk(name, spec):
        uops = lower(spec, ver="v3")
        sha = DveOpSpec(name=name, opcode=1, uops=uops,
                        rd1_en=dve_ops.has_src1(spec)).sha("v3")
        return dve_ops.DveOp(name, spec, subdim=False, uops_sha={"v3": sha})

    specs = {
        # h = t * q^2
        "CNL_MUL_SQ": Spec(
            body=Src0 * sq(Src1),
            reference=lambda in0, in1, c0, c1, c2: (
                in0.astype(np.float32) * np.square(in1.astype(np.float32))
            ),
        ),
        # z = (c0 - q)^2 * L
        "CNL_SQ1M_MUL": Spec(
            body=sq(C0 - Src0) * Src1,
            reference=lambda in0, in1, c0, c1, c2: (
                np.square(c0 - in0.astype(np.float32)) * in1.astype(np.float32)
            ),
        ),
        # acc = c1 + sum( ((y - c0)^2)^2 * z * imm2 )
        "CNL_QUART_ACC": Spec(
            body=sq(sq(Src0 - C0)) * Src1 * C2,
            accum=_op_add,
            accum_init=C1,
            reference=lambda in0, in1, c0, c1, c2: (
                lambda b: (b, c1 + b.reshape(b.shape[0], -1).sum(-1, keepdims=True))
            )((np.square(np.square(in0.astype(np.float32) - c0)) * in1.astype(np.float32) * c2).astype(np.float32)),
        ),
        # acc = c1 + sum( (c0 < y) ? h + imm2 : 0 )  — pos-masked sum with the
        # positive count encoded at offset imm2 (=BIG) in the same accumulator
        "CNL_SELBIG_ACC": Spec(
            body=select(C0 < Src1, Src0 + C2, Zero),
            accum=_op_add,
            accum_init=C1,
            reference=lambda in0, in1, c0, c1, c2: (
                lambda b: (b, c1 + b.reshape(b.shape[0], -1).sum(-1, keepdims=True))
            )(np.where(c0 < in1.astype(np.float32), in0.astype(np.float32) + c2, 0.0).astype(np.float32)),
        ),
    }
    existing = {op.name for op in dve_ops.OPS}
    made = {}
    for name, spec in specs.items():
        if name not in existing:
            op = _mk(name, spec)
            dve_ops.OPS.append(op)
        else:
            op = next(o for o in dve_ops.OPS if o.name == name)
        made[name] = op
    # rebuild the name->row map and CoreSim spec registry
    dve_ops._SUB_OPCODE_FOR_NAME.clear()
    dve_ops._SUB_OPCODE_FOR_NAME.update(
        {op.name: dve_ops._CUSTOM_DVE_ROW_BASE + i for i, op in enumerate(dve_ops.OPS)}
    )
    assert max(dve_ops._SUB_OPCODE_FOR_NAME.values()) < 0x20
    dve_ops.CUSTOM_DVE_SPECS.update({op.name: op.spec for op in dve_ops.OPS})
    made["TTR"] = next(o for o in dve_ops.OPS if o.name == "TENSOR_TENSOR_REDUCE")
    _custom_ops_cache.update(made)
    return _custom_ops_cache


BIG = 32768.0  # np-count encoding offset for the fused pos-select op


def build_module(b_pc=B_PC, c_maps=C, tile_f=2048, trn_type="TRN2",
                 skip_focal=False, skip_small=False, small_stage=6, reps=1):
    """Build the per-core Bass module (v7 pipeline). Returns nc."""
    ops = _get_custom_ops()

    nc = bacc.Bacc(trn_type, target_bir_lowering=False)
    nf = b_pc * c_maps * HWPLANE // 128
    n_tiles = nf // tile_f
    assert n_tiles * tile_f == nf

    maps = ["tl", "br", "ct"]
    hm, gt, embd, regs, gtr, inds = {}, {}, {}, {}, {}, {}
    for s in maps:
        hm[s] = nc.declare_dram_parameter(f"hmap_{s}", [128, nf], F32, isOutput=False)
        gt[s] = nc.declare_dram_parameter(f"gt_hmap_{s}", [128, nf], F32, isOutput=False)
    for s in ("tl", "br"):
        embd[s] = nc.declare_dram_parameter(f"embd_{s}", [b_pc, 128, 128], F32, isOutput=False)
    for s in maps:
        regs[s] = nc.declare_dram_parameter(f"regs_{s}", [b_pc, 2, 128, 128], F32, isOutput=False)
        gtr[s] = nc.declare_dram_parameter(f"gt_regs_{s}", [b_pc, 128, 2], F32, isOutput=False)
        inds[s] = nc.declare_dram_parameter(f"inds_{s}", [b_pc, 128, 1], I32, isOutput=False)
    masks_p = nc.declare_dram_parameter("ind_masks", [b_pc, 128, 1], I32, isOutput=False)
    out_p = nc.declare_dram_parameter("out", [128, RES_COLS], F32, isOutput=True)

    from contextlib import ExitStack

    with tile.TileContext(nc) as tc, ExitStack() as ctx:
        const_pool = ctx.enter_context(tc.tile_pool(name="const", bufs=1))
        acc_pool = ctx.enter_context(tc.tile_pool(name="acc", bufs=1))
        big_pool = ctx.enter_context(tc.tile_pool(name="big", bufs=3))
        # bf16 intermediates halve the mid-pool footprint — spend it on a
        # third buffer set for deeper DMA/ACT/DVE/GpSimd overlap
        mid_pool = ctx.enter_context(tc.tile_pool(name="mid", bufs=3))
        small_pool = ctx.enter_context(tc.tile_pool(name="small", bufs=2))
        psum_pool = ctx.enter_context(tc.tile_pool(name="ps", bufs=2, space="PSUM"))

        # ---- constants (loop-invariant) --------------------------------
        iota_fi = const_pool.tile([128, 128], I32, tag="iota_fi", name="iota_fi")
        nc.gpsimd.iota(iota_fi[:], pattern=[[1, 128]], base=0, channel_multiplier=0)
        iota_pi = const_pool.tile([128, 1], I32, tag="iota_pi", name="iota_pi")
        nc.gpsimd.iota(iota_pi[:], pattern=[[1, 1]], base=0, channel_multiplier=1)
        iota_f = const_pool.tile([128, 128], F32, tag="iota_f", name="iota_f")
        nc.vector.tensor_copy(iota_f[:], iota_fi[:])
        iota_p = const_pool.tile([128, 1], F32, tag="iota_p", name="iota_p")
        nc.vector.tensor_copy(iota_p[:], iota_pi[:])
        ident = const_pool.tile([128, 128], F32, tag="ident", name="ident")
        nc.vector.tensor_scalar(ident[:], iota_f[:], iota_p[:], None, ALU.is_equal)
        ones_row = const_pool.tile([1, 128], F32, tag="ones_row", name="ones_row")
        nc.vector.memset(ones_row[:], 1.0)
        ones2 = const_pool.tile([128, 2], F32, tag="ones2", name="ones2")
        nc.vector.memset(ones2[:], 1.0)
        cneg1 = const_pool.tile([128, 1], F32, tag="cneg1", name="cneg1")
        nc.vector.memset(cneg1[:], -1.0)

        def _run_body():
            results = acc_pool.tile([128, RES_COLS], F32, tag="results", name="results")
            nc.vector.memset(results[:], 0.0)

            # ---- focal loss over the 3 heatmap pairs -------------------
            accS = {}
            accP = {}
            for m in range(3):
                accS[m] = [acc_pool.tile([128, 1], F32, tag=f"accS{m}_{i}", name=f"accS{m}_{i}") for i in range(2)]
                nc.vector.memset(accS[m][0][:], 0.0)
                accP[m] = [acc_pool.tile([128, 1], F32, tag=f"accP{m}_{i}", name=f"accP{m}_{i}") for i in range(2)]
                nc.vector.memset(accP[m][0][:], 0.0)

            for m, s in enumerate(maps):
                if skip_focal:
                    break
                for j in range(n_tiles):
                    sl = slice(j * tile_f, (j + 1) * tile_f)
                    xt = big_pool.tile([128, tile_f], F32, tag="xt", name="xt")
                    nc.sync.dma_start(out=xt[:], in_=hm[s][:, sl])
                    yt = big_pool.tile([128, tile_f], F32, tag="yt", name="yt")
                    nc.sync.dma_start(out=yt[:], in_=gt[s][:, sl])

                    et = mid_pool.tile([128, tile_f], F32, tag="et", name="et")
                    nc.scalar.activation(et[:], xt[:], AF.Exp)
                    Lt = mid_pool.tile([128, tile_f], BF16, tag="Lt", name="Lt")
                    nc.scalar.activation(Lt[:], et[:], AF.Ln, bias=1.0)
                    qt = mid_pool.tile([128, tile_f], BF16, tag="qt", name="qt")
                    nc.scalar.activation(qt[:], Lt[:], AF.Exp, scale=-1.0)

                    # t = x - L on the otherwise idle GpSimd engine; VectorE
                    # is the bottleneck (custom ops are 1x-rate only)
                    tt = mid_pool.tile([128, tile_f], BF16, tag="tt", name="tt")
                    nc.gpsimd.tensor_tensor(tt[:], xt[:], Lt[:], ALU.subtract)

                    ht = mid_pool.tile([128, tile_f], BF16, tag="ht", name="ht")
                    nc.vector._custom_dve(ops["CNL_MUL_SQ"], out=ht[:], in0=tt[:], in1=qt[:])
                    # p^2 = Square(q - 1) on ACT, then z = p2 * L as a
                    # native bf16 tensor_tensor (2x mode) — keeps the
                    # 1x-rate SQ1M custom op off the DVE critical path
                    p2t = mid_pool.tile([128, tile_f], BF16, tag="p2t", name="p2t")
                    nc.scalar.activation(p2t[:], qt[:], AF.Square, bias=cneg1[:])
                    zt = mid_pool.tile([128, tile_f], BF16, tag="zt", name="zt")
                    nc.vector.tensor_tensor(zt[:], p2t[:], Lt[:], ALU.mult)
                    sA = mid_pool.tile([128, tile_f], BF16, tag="sA", name="sA")
                    nc.vector._custom_dve(
                        ops["CNL_SELBIG_ACC"], out=sA[:], in0=ht[:], in1=yt[:],
                        s0=0.999, s1=accP[m][j % 2][:], imm2=BIG,
                        accum_out=accP[m][(j + 1) % 2][:],
                    )
                    sB = mid_pool.tile([128, tile_f], BF16, tag="sB", name="sB")
                    nc.vector._custom_dve(
                        ops["CNL_QUART_ACC"], out=sB[:], in0=yt[:], in1=zt[:],
                        s0=1.0, s1=accS[m][j % 2][:], imm2=-1.0,
                        accum_out=accS[m][(j + 1) % 2][:],
                    )
                nc.vector.tensor_copy(results[:, RC_S[m]:RC_S[m] + 1], accS[m][n_tiles % 2][:])
                nc.vector.tensor_copy(results[:, RC_NP[m]:RC_NP[m] + 1], accP[m][n_tiles % 2][:])

            # ---- gathers + smooth-L1 + AE ------------------------------
            accL1 = [acc_pool.tile([128, 1], F32, tag=f"accL1_{i}", name=f"accL1_{i}") for i in range(2)]
            nc.vector.memset(accL1[0][:], 0.0)
            l1_step = 0

            for b in range(b_pc if not skip_small else 0):
                mask_i = small_pool.tile([128, 1], I32, tag="mask_i", name="mask_i")
                nc.sync.dma_start(out=mask_i[:], in_=masks_p[b])
                mask_f = small_pool.tile([128, 1], F32, tag="mask_f", name="mask_f")
                nc.vector.tensor_copy(mask_f[:], mask_i[:])

                ae_in = small_pool.tile([128, 4], F32, tag="ae_in", name="ae_in")
                nc.vector.memset(ae_in[:], 0.0)
                nc.vector.tensor_copy(ae_in[:, 2:3], mask_f[:])

                if small_stage < 2:
                    continue
                for m, s in enumerate(maps):
                    has_embd = s in ("tl", "br")
                    ind_t = small_pool.tile([128, 1], I32, tag="ind_t", name="ind_t")
                    nc.sync.dma_start(out=ind_t[:], in_=inds[s][b])
                    r_ti = small_pool.tile([128, 1], I32, tag="r_ti", name="r_ti")
                    nc.vector.tensor_scalar(r_ti[:], ind_t[:], 7, None, ALU.arith_shift_right)
                    c_ti = small_pool.tile([128, 1], I32, tag="c_ti", name="c_ti")
                    nc.vector.tensor_scalar(c_ti[:], ind_t[:], 127, None, ALU.bitwise_and)
                    r_t = small_pool.tile([128, 1], F32, tag="r_t", name="r_t")
                    nc.vector.tensor_copy(r_t[:], r_ti[:])
                    c_t = small_pool.tile([128, 1], F32, tag="c_t", name="c_t")
                    nc.vector.tensor_copy(c_t[:], c_ti[:])
                    ohRT = small_pool.tile([128, 128], F32, tag="ohRT", name="ohRT")
                    nc.vector.tensor_scalar(ohRT[:], iota_f[:], r_t[:], None, ALU.is_equal)
                    ohCT = small_pool.tile([128, 128], F32, tag="ohCT", name="ohCT")
                    nc.vector.tensor_scalar(ohCT[:], iota_f[:], c_t[:], None, ALU.is_equal)
                    if small_stage < 3:
                        continue
                    psT = psum_pool.tile([128, 128], F32, tag="psT", name="psT")
                    nc.tensor.transpose(psT[:], ohRT[:], ident[:])
                    ohR = small_pool.tile([128, 128], F32, tag="ohR", name="ohR")
                    nc.vector.tensor_copy(ohR[:], psT[:])

                    if small_stage < 4:
                        continue
                    gcols = 384 if has_embd else 256
                    off = 128 if has_embd else 0
                    plane = small_pool.tile([128, 384], F32, tag="plane", name="plane")
                    if has_embd:
                        nc.sync.dma_start(out=plane[:, 0:128], in_=embd[s][b])
                    nc.sync.dma_start(out=plane[:, off:off + 128], in_=regs[s][b, 0])
                    nc.sync.dma_start(out=plane[:, off + 128:off + 256], in_=regs[s][b, 1])

                    psG = psum_pool.tile([128, 384], F32, tag="psG", name="psG")
                    nc.tensor.matmul(psG[:, 0:gcols], lhsT=ohR[:], rhs=plane[:, 0:gcols],
                                     start=True, stop=True)

                    scr = small_pool.tile([128, 128], F32, tag="scr", name="scr")
                    if has_embd:
                        col = 0 if s == "tl" else 1
                        nc.vector._custom_dve(
                            ops["TTR"], out=scr[:], in0=psG[:, 0:128], in1=ohCT[:],
                            s0=0.0, s1=1.0, accum_out=ae_in[:, col:col + 1],
                        )
                    rg = small_pool.tile([128, 2], F32, tag="rg", name="rg")
                    for ch in range(2):
                        o = off + ch * 128
                        nc.vector._custom_dve(
                            ops["TTR"], out=scr[:], in0=psG[:, o:o + 128], in1=ohCT[:],
                            s0=0.0, s1=1.0, accum_out=rg[:, ch:ch + 1],
                        )

                    if small_stage < 5:
                        continue
                    gtg = small_pool.tile([128, 2], F32, tag="gtg", name="gtg")
                    nc.sync.dma_start(out=gtg[:], in_=gtr[s][b])
                    d = small_pool.tile([128, 2], F32, tag="d", name="d")
                    nc.vector.tensor_tensor(d[:], rg[:], gtg[:], ALU.subtract)
                    ad = small_pool.tile([128, 2], F32, tag="ad", name="ad")
                    nc.scalar.activation(ad[:], d[:], AF.Abs)
                    mn = small_pool.tile([128, 2], F32, tag="mn", name="mn")
                    nc.vector.tensor_scalar(mn[:], ad[:], 1.0, None, ALU.min)
                    t1 = small_pool.tile([128, 2], F32, tag="t1", name="t1")
                    nc.vector.tensor_scalar(t1[:], ad[:], 2.0, None, ALU.mult)
                    w1 = small_pool.tile([128, 2], F32, tag="w1", name="w1")
                    nc.vector.tensor_tensor(w1[:], t1[:], mn[:], ALU.subtract)
                    smk = small_pool.tile([128, 2], F32, tag="smk", name="smk")
                    nc.vector.tensor_scalar(smk[:], w1[:], mask_f[:], None, ALU.mult)
                    sm = small_pool.tile([128, 2], F32, tag="sm", name="sm")
                    nc.vector.tensor_tensor(sm[:], smk[:], mn[:], ALU.mult)
                    scr2 = small_pool.tile([128, 2], F32, tag="scr2", name="scr2")
                    nc.vector._custom_dve(
                        ops["TTR"], out=scr2[:], in0=sm[:], in1=ones2[:],
                        s0=accL1[l1_step % 2][:], s1=0.5,
                        accum_out=accL1[(l1_step + 1) % 2][:],
                    )
                    l1_step += 1

                if small_stage < 6:
                    continue
                # ---- AE loss for this batch item ----------------------
                # transpose e0/e1/mask columns into rows of a single-partition
                # [1, 384] psum tile (PSUM reads must start at partition 0)
                psA = psum_pool.tile([1, 384], F32, tag="psA", name="psA")
                for ci in range(3):
                    nc.tensor.transpose(psA[0:1, ci * 128:(ci + 1) * 128],
                                        ae_in[:, ci:ci + 1], ident[:])
                aeR = small_pool.tile([1, 384], F32, tag="aeR", name="aeR")
                nc.vector.tensor_copy(aeR[:], psA[:])
                s_row = small_pool.tile([1, 128], F32, tag="s_row", name="s_row")
                nc.vector.tensor_tensor(s_row[:], aeR[0:1, 0:128], aeR[0:1, 128:256], ALU.add)
                bc_in = small_pool.tile([1, 256], F32, tag="bc_in", name="bc_in")
                nc.vector.tensor_scalar(bc_in[0:1, 0:128], s_row[:], 0.5, None, ALU.mult)
                nc.vector.tensor_copy(bc_in[0:1, 128:256], aeR[0:1, 256:384])
                d_row = small_pool.tile([1, 128], F32, tag="d_row", name="d_row")
                nc.vector.tensor_tensor(d_row[:], aeR[0:1, 0:128], aeR[0:1, 128:256], ALU.subtract)
                sc = small_pool.tile([128, 1], F32, tag="sc", name="sc")
                nc.vector.tensor_tensor(sc[:], ae_in[:, 0:1], ae_in[:, 1:2], ALU.add)
                mc = small_pool.tile([128, 1], F32, tag="mc", name="mc")
                nc.vector.tensor_scalar(mc[:], sc[:], 0.5, None, ALU.mult)

                psB = psum_pool.tile([128, 256], F32, tag="psB", name="psB")
                nc.tensor.matmul(psB[:], lhsT=ones_row[:], rhs=bc_in[:], start=True, stop=True)

                Mm = small_pool.tile([128, 128], F32, tag="Mm", name="Mm")
                nc.vector.tensor_scalar(Mm[:], psB[:, 0:128], mc[:], None, ALU.subtract)
                Aa = small_pool.tile([128, 128], F32, tag="Aa", name="Aa")
                nc.scalar.activation(Aa[:], Mm[:], AF.Abs)
                Dd = small_pool.tile([128, 128], F32, tag="Dd", name="Dd")
                nc.vector.tensor_scalar(Dd[:], Aa[:], -1.0, 1.0, ALU.mult, ALU.add)
                pm = small_pool.tile([128, 128], F32, tag="pm", name="pm")
                nc.vector.tensor_scalar(pm[:], psB[:, 128:256], mask_f[:], None, ALU.mult)
                Dr = small_pool.tile([128, 128], F32, tag="Dr", name="Dr")
                nc.vector.tensor_scalar(Dr[:], Dd[:], 0.0, None, ALU.max)
                scrP = small_pool.tile([128, 128], F32, tag="scrP", name="scrP")
                nc.vector._custom_dve(
                    ops["TTR"], out=scrP[:], in0=Dr[:], in1=pm[:],
                    s0=0.0, s1=1.0,
                    accum_out=results[:, RC_S1 + b:RC_S1 + b + 1],
                )
                scrQ = small_pool.tile([128, 128], F32, tag="scrQ", name="scrQ")
                nc.vector.tensor_scalar(
                    scrQ[:], pm[:], 0.0, 0.0, ALU.add, ALU.add,
                    accum_out=results[:, RC_S2 + b:RC_S2 + b + 1],
                )
                sdm = small_pool.tile([1, 128], F32, tag="sdm", name="sdm")
                nc.vector.tensor_tensor(sdm[:], d_row[:], bc_in[0:1, 128:256], ALU.mult)
                scrR = small_pool.tile([1, 128], F32, tag="scrR", name="scrR")
                nc.vector._custom_dve(
                    ops["TTR"], out=scrR[:], in0=sdm[:], in1=d_row[:],
                    s0=0.0, s1=0.5,
                    accum_out=results[0:1, RC_PULL + b:RC_PULL + b + 1],
                )
                scrN = small_pool.tile([1, 128], F32, tag="scrN", name="scrN")
                nc.vector.tensor_scalar(
                    scrN[:], bc_in[0:1, 128:256], 0.0, 0.0, ALU.add, ALU.add,
                    accum_out=results[0:1, RC_NUM + b:RC_NUM + b + 1],
                )

            nc.vector.tensor_copy(results[:, RC_L1:RC_L1 + 1], accL1[l1_step % 2][:])
            nc.sync.dma_start(out=out_p[:], in_=results[:])

        if reps > 1:
            with tc.For_i(0, reps):
                _run_body()
        else:
            _run_body()

    nc.compile()
    return nc


def shard_inputs(inputs, b_pc=B_PC, n_cores=N_CORES):
    """Slice full inputs into per-core input maps (named as dram params)."""
    in_maps = []
    for core in range(n_cores):
        b0, b1 = core * b_pc, (core + 1) * b_pc
        m = {}
        for s in ("tl", "br", "ct"):
            m[f"hmap_{s}"] = np.ascontiguousarray(inputs[f"hmap_{s}"][b0:b1]).reshape(128, -1)
            m[f"gt_hmap_{s}"] = np.ascontiguousarray(inputs[f"gt_hmap_{s}"][b0:b1]).reshape(128, -1)
            m[f"regs_{s}"] = np.ascontiguousarray(inputs[f"regs_{s}"][b0:b1]).reshape(b_pc, 2, 128, 128)
            m[f"gt_regs_{s}"] = np.ascontiguousarray(inputs[f"gt_regs_{s}"][b0:b1]).reshape(b_pc, 128, 2)
            m[f"inds_{s}"] = np.ascontiguousarray(inputs[f"inds_{s}"][b0:b1]).reshape(b_pc, 128, 1)
        for s in ("tl", "br"):
            m[f"embd_{s}"] = np.ascontiguousarray(inputs[f"embd_{s}"][b0:b1]).reshape(b_pc, 128, 128)
        m["ind_masks"] = np.ascontiguousarray(inputs["ind_masks"][b0:b1]).reshape(b_pc, 128, 1)
        in_maps.append(m)
    return in_maps


def host_combine(res_list, b_pc=B_PC, fused_np=True):
    """Combine per-core [128,16] partial-sum tiles into the scalar loss."""
    res = [np.asarray(r, dtype=np.float64) for r in res_list]
    focal = 0.0
    for m in range(3):
        if fused_np:
            v = np.concatenate([r[:, RC_NP[m]] for r in res])
            np_p = np.round(v / BIG)
            PL = (v - np_p * BIG).sum()
            NP = np_p.sum()
            S = PL + sum(r[:, RC_S[m]].sum() for r in res)
        else:
            S = sum(r[:, RC_S[m]].sum() for r in res)
            NP = sum(r[:, RC_NP[m]].sum() for r in res)
        focal += (-S / NP) if NP > 0 else -S
    l1 = sum(r[:, RC_L1].sum() for r in res)
    num_global = 0.0
    pull = 0.0
    push = 0.0
    for r in res:
        for b in range(b_pc):
            num_b = r[0, RC_NUM + b]
            num_global += num_b
            pull += r[0, RC_PULL + b] / (num_b + EPS)
            S1 = r[:, RC_S1 + b].sum()
            S2 = r[:, RC_S2 + b].sum()
            push += (S1 - S2 / (num_b + EPS)) / ((num_b - 1.0) * num_b + EPS)
    reg = l1 / (num_global + EPS)
    total = focal + 0.1 * pull + 0.1 * push + reg
    return np.float32(total)


_nc_cache = {}
last_exec_time_ns = None

VERSION = 7  # host_combine fused_np path


def kernel(**inputs):
    global last_exec_time_ns
    if "nc" not in _nc_cache:
        _nc_cache["nc"] = build_module()
    nc = _nc_cache["nc"]
    in_maps = shard_inputs(inputs)
    trace = bool(int(os.environ.get("CNL_TRACE", "0")))
    kr = run_bass_kernel_spmd(nc, in_maps, core_ids=list(range(N_CORES)), trace=trace)
    last_exec_time_ns = kr.exec_time_ns
    return host_combine([r["out"] for r in kr.results], fused_np=True)
